# revision 2
# baseline (speedup 1.0000x reference)
"""Overlapping-chunk extraction kernel for Trainium2 (Bass).

Computes out[b, j, c, f] = x[b, 125*j + c, f] for j in [0, 255), c in [0, 250),
i.e. 255 half-overlapping chunks of length 250 from a (16, 32000, 64) signal.

Strategy (pure data movement, memory-bound):
  - Shard batch across 8 cores: 2 samples per core.
  - Stage each input sample in SBUF as [128 partitions x 16000 fp32]
    (partition p = frames [250p, 250p+250)), so each input byte is read
    from HBM exactly ONCE (the direct HBM->HBM variant reads the
    overlapping source twice). Per-core HBM traffic drops from 65.3 MB
    to 49.0 MB.
  - Outbound, per sample, 3 DMAs with large contiguous SBUF-side reads:
      * even chunks  = full partitions        (128 x 64 KB)
      * odd chunks' first halves  = partition k  bytes [32000:64000)  (127 x 32 KB)
      * odd chunks' second halves = partition k+1 bytes [0:32000)     (127 x 32 KB)
    HBM-side writes are strided (stride 128 KB) but each block is >=32 KB,
    well past descriptor-efficiency knee.
  - Inbound DMAs issue on the Sync HWDGE ring, outbound on the Scalar
    HWDGE ring; sample 1's load overlaps sample 0's stores.
"""

import numpy as np

import concourse.bass as bass
import concourse.mybir as mybir
from concourse.bass_utils import run_bass_kernel_spmd

# Problem shape (hardcoded per contract)
B, T, F = 16, 32000, 64
N_CORES = 8
S = B // N_CORES          # samples per core = 2
NFC = 128                 # non-overlapping chunks per sample
CHUNK = 250               # frames per chunk
NOV = 2 * NFC - 1         # 255 overlapped output chunks
PART_FREE = CHUNK * F     # 16000 fp32 per chunk (one SBUF partition row)
HALF_FREE = PART_FREE // 2  # 8000 fp32 = 125 frames (chunk advance)
SAMPLE_IN = T * F         # 2_048_000 fp32 per input sample
SAMPLE_OUT = NOV * PART_FREE  # 4_080_000 fp32 per output sample

_NC_CACHE = {}


def build_module(repeat=1, name="chunkop"):
    """Build the kernel program; `repeat` chains the whole kernel R times
    back-to-back (semaphore-gated) for HW timing via differencing."""
    nc = bass.Bass(trn_type="TRN2", name=name)
    x = nc.dram_tensor("x", [S, T, F], mybir.dt.float32, kind="ExternalInput")
    y = nc.dram_tensor(
        "y", [S, NOV, CHUNK, F], mybir.dt.float32, kind="ExternalOutput"
    )
    x_t = x[:, :, :].tensor
    y_t = y[:, :, :, :].tensor

    with (
        nc.sbuf_tensor([NFC, PART_FREE], mybir.dt.float32) as buf0,
        nc.sbuf_tensor([NFC, PART_FREE], mybir.dt.float32) as buf1,
        nc.semaphore("s_in") as s_in,
        nc.semaphore("s_out0") as s_out0,
        nc.semaphore("s_out1") as s_out1,
        nc.Block() as block,
    ):
        bufs = [buf0, buf1]
        s_outs = [s_out0, s_out1]

        @block.sync
        def _(sync):
            with nc.allow_non_contiguous_dma(reason="overlap chunk copies"):
                for r in range(repeat):
                    for s in range(S):
                        if r > 0:
                            # buffer s must be fully stored before reload
                            sync.wait_ge(s_outs[s], 48 * r)
                        src = bass.AP(
                            x_t, s * SAMPLE_IN, [[PART_FREE, NFC], [1, PART_FREE]]
                        )
                        sync.dma_start(bufs[s][:, :], src).then_inc(s_in, 16)

        @block.scalar
        def _(scalar):
            with nc.allow_non_contiguous_dma(reason="overlap chunk copies"):
                for r in range(repeat):
                    for s in range(S):
                        scalar.wait_ge(s_in, 32 * r + 16 * (s + 1))
                        base = s * SAMPLE_OUT
                        buf = bufs[s]
                        # even chunks j=2p: y[32000p : 32000p+16000) = partition p
                        dst = bass.AP(
                            y_t, base, [[2 * PART_FREE, NFC], [1, PART_FREE]]
                        )
                        scalar.dma_start(dst, buf[:, :]).then_inc(s_outs[s], 16)
                        # odd chunks j=2k+1, first half: partition k [8000:16000)
                        dst = bass.AP(
                            y_t,
                            base + PART_FREE,
                            [[2 * PART_FREE, NFC - 1], [1, HALF_FREE]],
                        )
                        scalar.dma_start(
                            dst, buf[0 : NFC - 1, HALF_FREE:PART_FREE]
                        ).then_inc(s_outs[s], 16)
                        # odd chunks j=2k+1, second half: partition k+1 [0:8000)
                        dst = bass.AP(
                            y_t,
                            base + PART_FREE + HALF_FREE,
                            [[2 * PART_FREE, NFC - 1], [1, HALF_FREE]],
                        )
                        scalar.dma_start(
                            dst, buf[1:NFC, 0:HALF_FREE]
                        ).then_inc(s_outs[s], 16)
                scalar.wait_ge(s_out0, 48 * repeat)
                scalar.wait_ge(s_out1, 48 * repeat)

    return nc


def get_module():
    if "nc" not in _NC_CACHE:
        _NC_CACHE["nc"] = build_module()
    return _NC_CACHE["nc"]


def kernel(x):
    x = np.ascontiguousarray(np.asarray(x), dtype=np.float32)
    assert x.shape == (B, T, F), x.shape
    nc = get_module()
    in_maps = [{"x": x[i * S : (i + 1) * S]} for i in range(N_CORES)]
    res = run_bass_kernel_spmd(nc, in_maps, core_ids=list(range(N_CORES)))
    return np.concatenate([r["y"] for r in res.results], axis=0)
